# revision 1
# baseline (speedup 1.0000x reference)
"""ChebConv layer (B=128, N=512, F=32, K=3) on 8 TRN2 NeuronCores.

Math: with lambda_max = 2.0 the scaled Laplacian collapses to Lhat = -Ahat,
Ahat = D^-1/2 A D^-1/2.  Folding the degree scalings into the vectors:
    u  = A (dinv*x)          Ahat x        = dinv*u
    v  = A (dinv^2 * u)      Ahat Ahat x   = dinv*v
    out = relu( x(W0-W2) + (dinv*u)(-W1) + (dinv*v)(2 W2) + b ) + x

Sharding: data-parallel over batch, 16 samples per core, no collectives.
Host preps per-shard layout: adj and x transposed per sample so the device
reads adjT[m, n] with the contraction index m on SBUF partitions; the device
computes everything in [f, n] space and returns out^T, un-transposed on host.

Partition placement: xT/zT/acc/oT live on partitions 0-31, u^T on 32-63
(matmul col-group 1), v^T on 64-95 (col-group 2) so every vector op is
lane-aligned and the epilogue is a single K=96 stacked matmul.
"""

import os
import sys

sys.path.insert(0, "/opt/trn_rl_repo")

import numpy as np

import concourse.bass as bass
from concourse import bacc
import concourse.mybir as mybir
import concourse.tile as tile
from concourse.bass_utils import run_bass_kernel_spmd
from contextlib import ExitStack

B, N, F = 128, 512, 32
NCORES = 8
S = B // NCORES          # samples per core
P = 128                  # SBUF partitions
C = N // P               # m-chunks per sample (4)

f32 = mybir.dt.float32
bf16 = mybir.dt.bfloat16

_cache = {}


def _install_ntff_hook():
    """Provide antenv.axon_hooks (missing in this image) so trace=True works."""
    import contextlib
    import ctypes
    import types

    try:
        from antenv.axon_hooks import get_axon_ntff_profile_hook  # noqa: F401
        return
    except ImportError:
        pass
    so_path = "/opt/axon/libaxon_pjrt.so"
    if not os.path.exists(so_path):
        return
    lib = ctypes.CDLL(so_path)
    if not hasattr(lib, "axon_start_nrt_profile"):
        return
    lib.axon_start_nrt_profile.argtypes = [
        ctypes.POINTER(ctypes.c_int64), ctypes.c_size_t,
    ]
    lib.axon_start_nrt_profile.restype = ctypes.c_int64
    lib.axon_stop_nrt_profile.argtypes = [ctypes.c_char_p]
    lib.axon_stop_nrt_profile.restype = ctypes.c_int64

    @contextlib.contextmanager
    def _hook(output_dir, device_ids):
        import jax

        jax.devices()
        if device_ids:
            ids = (ctypes.c_int64 * len(device_ids))(*device_ids)
            rc = lib.axon_start_nrt_profile(ids, len(device_ids))
        else:
            rc = lib.axon_start_nrt_profile(None, 0)
        if rc != 0:
            raise RuntimeError(f"axon_start_nrt_profile rc={rc}")
        try:
            yield
        finally:
            n = lib.axon_stop_nrt_profile(str(output_dir).encode())
            print(f"profile: {n} file(s) written to {output_dir}", file=sys.stderr)

    mod = types.ModuleType("antenv.axon_hooks")
    state = {"hook": _hook}
    mod.get_axon_ntff_profile_hook = lambda: state["hook"]
    mod.set_axon_ntff_profile_hook = lambda h: state.update(hook=h)
    sys.modules["antenv.axon_hooks"] = mod


def build_nc():
    nc = bacc.Bacc()
    adjT = nc.declare_dram_parameter("adjT", [S, N, N], f32, isOutput=False)
    xT = nc.declare_dram_parameter("xT", [S, F, N], bf16, isOutput=False)
    vs_d = nc.declare_dram_parameter("vs", [3 * F, F], bf16, isOutput=False)
    b_d = nc.declare_dram_parameter("bcol", [F, 1], f32, isOutput=False)
    id_d = nc.declare_dram_parameter("ident2", [2 * F, F], bf16, isOutput=False)
    out_d = nc.declare_dram_parameter("out", [S, F, N], f32, isOutput=True)

    with tile.TileContext(nc) as tc, ExitStack() as ctx:
        consts = ctx.enter_context(tc.tile_pool(name="consts", bufs=1))
        adj_pool = ctx.enter_context(tc.tile_pool(name="adj", bufs=10))
        stack_pool = ctx.enter_context(tc.tile_pool(name="stack", bufs=11))
        work = ctx.enter_context(tc.tile_pool(name="work", bufs=5))
        ps_tr = ctx.enter_context(tc.tile_pool(name="pstr", bufs=4, space="PSUM"))
        ps_big = ctx.enter_context(tc.tile_pool(name="psbig", bufs=4, space="PSUM"))

        ones = consts.tile([P, 1], bf16, tag="ones")
        nc.vector.memset(ones, 1.0)
        ident2 = consts.tile([2 * F, F], bf16, tag="ident2")
        nc.sync.dma_start(out=ident2, in_=id_d[:, :])
        vs = consts.tile([3 * F, F], bf16, tag="vs")
        nc.sync.dma_start(out=vs, in_=vs_d[:, :])
        bcol = consts.tile([F, 1], f32, tag="bcol")
        nc.sync.dma_start(out=bcol, in_=b_d[:, :])

        def stage_a(s):
            """Issue input DMAs."""
            at = adj_pool.tile([P, C, N], bf16, tag="adj")
            nc.gpsimd.dma_start(out=at, in_=adjT[s].rearrange("(p c) n -> p c n", p=P))
            stack = stack_pool.tile([3 * F, N], bf16, tag="stack")
            nc.sync.dma_start(out=stack[0:F, :], in_=xT[s])
            return {"at": at, "stack": stack}

        def stage_b(st):
            """Degree, dinv chain, zT (emitted at iteration end)."""
            at, stack = st["at"], st["stack"]
            ps = ps_big.tile([P, N], f32, tag="big")
            st["ps"] = ps
            deg = ps[0:1, :]
            for c in range(C):
                nc.tensor.matmul(
                    deg, ones, at[:, c, :], start=(c == 0), stop=(c == C - 1),
                )
            sq = work.tile([1, N], f32, tag="sq")
            nc.scalar.activation(out=sq, in_=deg, func=mybir.ActivationFunctionType.Sqrt)
            dinvf = work.tile([1, N], f32, tag="dinvf")
            nc.vector.reciprocal_approx_fast(out=dinvf, in_=sq)
            dinvb = work.tile([1, N], bf16, tag="dinvb")
            nc.vector.tensor_copy(out=dinvb, in_=dinvf)
            dinv96 = work.tile([3 * F, N], bf16, tag="dinv96")
            nc.gpsimd.partition_broadcast(dinv96, dinvb)
            zT = work.tile([F, N], bf16, tag="zT")
            nc.vector.tensor_mul(zT, stack[0:F, :], dinv96[0:F, :])
            st.update(dinv96=dinv96, zT=zT)

        def stage_c(st):
            """z transposes, zn copy, u matmuls, duT and y1T scales."""
            zT = st["zT"]
            zTr = zT.rearrange("f (p c) -> f c p", c=C)
            znp = ps_tr.tile([P, C * F], bf16, tag="tr")
            for c in range(C):
                nc.tensor.transpose(
                    znp[:, c * F:(c + 1) * F], zTr[:, c, :], ident2[0:F, :]
                )
            zn = work.tile([P, C * F], bf16, tag="zn")
            nc.scalar.activation(out=zn, in_=znp, func=mybir.ActivationFunctionType.Copy)
            at, ps, stack, dinv96 = st["at"], st["ps"], st["stack"], st["dinv96"]
            uT = ps[F:2 * F, :]
            for c in range(C):
                nc.tensor.matmul(
                    uT, zn[:, c * F:(c + 1) * F], at[:, c, :],
                    start=(c == 0), stop=(c == C - 1), tile_position=(0, F),
                )
            nc.vector.tensor_mul(stack[F:2 * F, :], uT, dinv96[F:2 * F, :])
            y1T_t = work.tile([2 * F, N], bf16, tag="y1T")
            y1T = y1T_t[F:2 * F, :]
            nc.vector.tensor_mul(y1T, stack[F:2 * F, :], dinv96[F:2 * F, :])
            st["y1T"] = y1T

        def stage_d(st):
            """y1 transposes, v matmuls, dvT scale."""
            y1T, at, ps, stack, dinv96 = st["y1T"], st["at"], st["ps"], st["stack"], st["dinv96"]
            y1r = y1T.rearrange("f (p c) -> f c p", c=C)
            y1p = ps_tr.tile([P, C * F], bf16, tag="tr")
            for c in range(C):
                nc.tensor.transpose(
                    y1p[:, c * F:(c + 1) * F], y1r[:, c, :], ident2[F:2 * F, :]
                )
            y1n = work.tile([P, C * F], bf16, tag="y1n")
            nc.scalar.activation(out=y1n, in_=y1p, func=mybir.ActivationFunctionType.Copy)
            vT = ps[2 * F:3 * F, :]
            for c in range(C):
                nc.tensor.matmul(
                    vT, y1n[:, c * F:(c + 1) * F], at[:, c, :],
                    start=(c == 0), stop=(c == C - 1), tile_position=(0, 2 * F),
                )
            nc.vector.tensor_mul(stack[2 * F:3 * F, :], vT, dinv96[2 * F:3 * F, :])

        def stage_e(st, s):
            """Epilogue matmul, relu+bias, residual, DMA out."""
            ps, stack = st["ps"], st["stack"]
            acc = ps[0:F, :]
            nc.tensor.matmul(acc, vs, stack, start=True, stop=True)
            oT = work.tile([F, N], f32, tag="oT")
            nc.scalar.activation(
                out=oT, in_=acc, func=mybir.ActivationFunctionType.Relu,
                bias=bcol, scale=1.0,
            )
            nc.gpsimd.tensor_add(oT, oT, stack[0:F, :])
            nc.sync.dma_start(out=out_d[s], in_=oT)

        pipe = {}
        for s in range(min(5, S)):
            pipe[s] = stage_a(s)
        for i in range(S + 4):
            if i + 5 < S:
                pipe[i + 5] = stage_a(i + 5)
            if 0 <= i - 2 < S:
                stage_c(pipe[i - 2])
            if 0 <= i - 3 < S:
                stage_d(pipe[i - 3])
            if 0 <= i - 4 < S:
                stage_e(pipe[i - 4], i - 4)
                del pipe[i - 4]["ps"]
            if 0 <= i - 1 < S:
                stage_b(pipe[i - 1])

    nc.finalize()
    return nc


def kernel(adj, x, W, b):
    adj = np.ascontiguousarray(adj, dtype=np.float32)
    x = np.ascontiguousarray(x, dtype=np.float32)
    W = np.asarray(W, dtype=np.float32)
    b = np.asarray(b, dtype=np.float32)

    # fold the Chebyshev recursion constants into one stacked weight
    import ml_dtypes
    vs = np.concatenate([W[0] - W[2], -W[1], 2.0 * W[2]], axis=0).astype(
        ml_dtypes.bfloat16)  # [96, 32]
    bcol = b.reshape(F, 1)
    eye = np.eye(F, dtype=np.float32)
    ident2 = np.concatenate([eye, eye], axis=0).astype(ml_dtypes.bfloat16)  # [64, 32]

    if "nc" not in _cache:
        _cache["nc"] = build_nc()
    nc = _cache["nc"]

    in_maps = []
    for i in range(NCORES):
        sl = slice(i * S, (i + 1) * S)
        in_maps.append({
            "adjT": np.ascontiguousarray(adj[sl].transpose(0, 2, 1)),
            "xT": np.ascontiguousarray(x[sl].transpose(0, 2, 1)).astype(ml_dtypes.bfloat16),
            "vs": vs,
            "bcol": bcol,
            "ident2": ident2,
        })

    trace = os.environ.get("KERNEL_TRACE") == "1"
    kw = {}
    if trace:
        _install_ntff_hook()
        import concourse.bass_utils as _bu
        _bu.upload_artifacts = lambda t: t  # no bucket in this container
        kw["tmpdir"] = os.environ.get("KERNEL_TRACE_DIR") or None
    res = run_bass_kernel_spmd(
        nc, in_maps, core_ids=list(range(NCORES)), trace=trace, **kw,
    )
    if trace and res.exec_time_ns is not None:
        print(f"HW exec time: {res.exec_time_ns} ns")

    outT = np.concatenate([res.results[i]["out"] for i in range(NCORES)], axis=0)
    return np.ascontiguousarray(outT.transpose(0, 2, 1))



# revision 5
# speedup vs baseline: 1.4370x; 1.4370x over previous
"""ChebConv layer (B=128, N=512, F=32, K=3) on 8 TRN2 NeuronCores.

Math: with lambda_max = 2.0 the scaled Laplacian collapses to Lhat = -Ahat,
Ahat = D^-1/2 A D^-1/2.  Folding the degree scalings into the vectors:
    u  = A (dinv*x)          Ahat x        = dinv*u
    v  = A (dinv^2 * u)      Ahat Ahat x   = dinv*v
    out = relu( x(W0-W2) + (dinv*u)(-W1) + (dinv*v)(2 W2) + b ) + x

Sharding: data-parallel over batch, 16 samples per core, no collectives.
Host preps per-shard layout: adj and x transposed per sample so the device
reads adjT[m, n] with the contraction index m on SBUF partitions; the device
computes everything in [f, n] space and returns out^T, un-transposed on host.

Partition placement: xT/zT/acc/oT live on partitions 0-31, u^T on 32-63
(matmul col-group 1), v^T on 64-95 (col-group 2) so every vector op is
lane-aligned and the epilogue is a single K=96 stacked matmul.
"""

import os
import sys

sys.path.insert(0, "/opt/trn_rl_repo")

import numpy as np

import concourse.bass as bass
from concourse import bacc
import concourse.mybir as mybir
import concourse.tile as tile
from concourse.bass_utils import run_bass_kernel_spmd
from contextlib import ExitStack

B, N, F = 128, 512, 32
NCORES = 8
S = B // NCORES          # samples per core
P = 128                  # SBUF partitions
C = N // P               # m-chunks per sample (4)

f32 = mybir.dt.float32
bf16 = mybir.dt.bfloat16

_cache = {}


def _install_ntff_hook():
    """Provide antenv.axon_hooks (missing in this image) so trace=True works."""
    import contextlib
    import ctypes
    import types

    try:
        from antenv.axon_hooks import get_axon_ntff_profile_hook  # noqa: F401
        return
    except ImportError:
        pass
    so_path = "/opt/axon/libaxon_pjrt.so"
    if not os.path.exists(so_path):
        return
    lib = ctypes.CDLL(so_path)
    if not hasattr(lib, "axon_start_nrt_profile"):
        return
    lib.axon_start_nrt_profile.argtypes = [
        ctypes.POINTER(ctypes.c_int64), ctypes.c_size_t,
    ]
    lib.axon_start_nrt_profile.restype = ctypes.c_int64
    lib.axon_stop_nrt_profile.argtypes = [ctypes.c_char_p]
    lib.axon_stop_nrt_profile.restype = ctypes.c_int64

    @contextlib.contextmanager
    def _hook(output_dir, device_ids):
        import jax

        jax.devices()
        if device_ids:
            ids = (ctypes.c_int64 * len(device_ids))(*device_ids)
            rc = lib.axon_start_nrt_profile(ids, len(device_ids))
        else:
            rc = lib.axon_start_nrt_profile(None, 0)
        if rc != 0:
            raise RuntimeError(f"axon_start_nrt_profile rc={rc}")
        try:
            yield
        finally:
            n = lib.axon_stop_nrt_profile(str(output_dir).encode())
            print(f"profile: {n} file(s) written to {output_dir}", file=sys.stderr)

    mod = types.ModuleType("antenv.axon_hooks")
    state = {"hook": _hook}
    mod.get_axon_ntff_profile_hook = lambda: state["hook"]
    mod.set_axon_ntff_profile_hook = lambda h: state.update(hook=h)
    sys.modules["antenv.axon_hooks"] = mod


def build_nc():
    nc = bacc.Bacc()
    adjT = nc.declare_dram_parameter("adjT", [S, N, N], bf16, isOutput=False)
    xT = nc.declare_dram_parameter("xT", [S, F, N], bf16, isOutput=False)
    vs_d = nc.declare_dram_parameter("vs", [3 * F, F], bf16, isOutput=False)
    b_d = nc.declare_dram_parameter("bcol", [F, 1], f32, isOutput=False)
    id_d = nc.declare_dram_parameter("ident2", [2 * F, F], bf16, isOutput=False)
    out_d = nc.declare_dram_parameter("out", [S, F, N], f32, isOutput=True)

    with tile.TileContext(nc) as tc, ExitStack() as ctx:
        consts = ctx.enter_context(tc.tile_pool(name="consts", bufs=1))
        adj_pool = ctx.enter_context(tc.tile_pool(name="adj", bufs=10))
        stack_pool = ctx.enter_context(tc.tile_pool(name="stack", bufs=11))
        work = ctx.enter_context(tc.tile_pool(name="work", bufs=5))
        ps_tr = ctx.enter_context(tc.tile_pool(name="pstr", bufs=4, space="PSUM"))
        ps_big = ctx.enter_context(tc.tile_pool(name="psbig", bufs=4, space="PSUM"))

        ones = consts.tile([P, 1], bf16, tag="ones")
        nc.vector.memset(ones, 1.0)
        ident2 = consts.tile([2 * F, F], bf16, tag="ident2")
        nc.sync.dma_start(out=ident2, in_=id_d[:, :])
        vs = consts.tile([3 * F, F], bf16, tag="vs")
        nc.sync.dma_start(out=vs, in_=vs_d[:, :])
        bcol = consts.tile([F, 1], f32, tag="bcol")
        nc.sync.dma_start(out=bcol, in_=b_d[:, :])

        def stage_a(s):
            """Issue input DMAs."""
            at = adj_pool.tile([P, C, N], bf16, tag="adj")
            nc.sync.dma_start(out=at, in_=adjT[s].rearrange("(p c) n -> p c n", p=P))
            stack = stack_pool.tile([3 * F, N], bf16, tag="stack")
            nc.scalar.dma_start(out=stack[0:F, :], in_=xT[s])
            return {"at": at, "stack": stack}

        def stage_b(st):
            """Degree, dinv chain, zT (emitted at iteration end)."""
            at, stack = st["at"], st["stack"]
            ps = ps_big.tile([P, N], f32, tag="big")
            st["ps"] = ps
            deg = ps[0:1, :]
            for c in range(C):
                nc.tensor.matmul(
                    deg, ones, at[:, c, :], start=(c == 0), stop=(c == C - 1),
                )
            sq = work.tile([1, N], f32, tag="sq")
            nc.scalar.activation(out=sq, in_=deg, func=mybir.ActivationFunctionType.Sqrt)
            dinvf = work.tile([1, N], f32, tag="dinvf")
            nc.vector.reciprocal_approx_fast(out=dinvf, in_=sq)
            dinvb = work.tile([1, N], bf16, tag="dinvb")
            nc.vector.tensor_copy(out=dinvb, in_=dinvf)
            dinv96 = work.tile([3 * F, N], bf16, tag="dinv96")
            nc.gpsimd.partition_broadcast(dinv96, dinvb)
            zT = work.tile([F, N], bf16, tag="zT")
            nc.vector.tensor_mul(zT, stack[0:F, :], dinv96[0:F, :])
            st.update(dinv96=dinv96, zT=zT)

        def stage_c(st):
            """z transposes, zn copy, u matmuls, duT and y1T scales."""
            zT = st["zT"]
            zTr = zT.rearrange("f (p c) -> f c p", c=C)
            znp = ps_tr.tile([P, C * F], bf16, tag="tr")
            for c in range(C):
                nc.tensor.transpose(
                    znp[:, c * F:(c + 1) * F], zTr[:, c, :], ident2[0:F, :]
                )
            zn = work.tile([P, C * F], bf16, tag="zn")
            nc.scalar.activation(out=zn, in_=znp, func=mybir.ActivationFunctionType.Copy)
            at, ps, stack, dinv96 = st["at"], st["ps"], st["stack"], st["dinv96"]
            uT = ps[F:2 * F, :]
            for c in range(C):
                nc.tensor.matmul(
                    uT, zn[:, c * F:(c + 1) * F], at[:, c, :],
                    start=(c == 0), stop=(c == C - 1), tile_position=(0, F),
                )
            nc.vector.tensor_mul(stack[F:2 * F, :], uT, dinv96[F:2 * F, :])
            y1T_t = work.tile([2 * F, N], bf16, tag="y1T")
            y1T = y1T_t[F:2 * F, :]
            nc.vector.tensor_mul(y1T, stack[F:2 * F, :], dinv96[F:2 * F, :])
            st["y1T"] = y1T

        def stage_d(st):
            """y1 transposes, v matmuls, dvT scale."""
            y1T, at, ps, stack, dinv96 = st["y1T"], st["at"], st["ps"], st["stack"], st["dinv96"]
            y1r = y1T.rearrange("f (p c) -> f c p", c=C)
            y1p = ps_tr.tile([P, C * F], bf16, tag="tr")
            for c in range(C):
                nc.tensor.transpose(
                    y1p[:, c * F:(c + 1) * F], y1r[:, c, :], ident2[F:2 * F, :]
                )
            y1n = work.tile([P, C * F], bf16, tag="y1n")
            nc.scalar.activation(out=y1n, in_=y1p, func=mybir.ActivationFunctionType.Copy)
            vT = ps[2 * F:3 * F, :]
            for c in range(C):
                nc.tensor.matmul(
                    vT, y1n[:, c * F:(c + 1) * F], at[:, c, :],
                    start=(c == 0), stop=(c == C - 1), tile_position=(0, 2 * F),
                )
            nc.vector.tensor_mul(stack[2 * F:3 * F, :], vT, dinv96[2 * F:3 * F, :])

        def stage_e(st, s):
            """Epilogue matmul, relu+bias, residual, DMA out."""
            ps, stack = st["ps"], st["stack"]
            acc = ps[0:F, :]
            nc.tensor.matmul(acc, vs, stack, start=True, stop=True)
            oT = work.tile([F, N], f32, tag="oT")
            nc.scalar.activation(
                out=oT, in_=acc, func=mybir.ActivationFunctionType.Relu,
                bias=bcol, scale=1.0,
            )
            nc.vector.tensor_add(oT, oT, stack[0:F, :])
            nc.scalar.dma_start(out=out_d[s], in_=oT)

        pipe = {}
        for s in range(min(5, S)):
            pipe[s] = stage_a(s)
        for i in range(S + 4):
            if i + 5 < S:
                pipe[i + 5] = stage_a(i + 5)
            if 0 <= i - 2 < S:
                stage_c(pipe[i - 2])
            if 0 <= i - 3 < S:
                stage_d(pipe[i - 3])
            if 0 <= i - 4 < S:
                stage_e(pipe[i - 4], i - 4)
                del pipe[i - 4]["ps"]
            if 0 <= i - 1 < S:
                stage_b(pipe[i - 1])

    nc.finalize()
    return nc


def kernel(adj, x, W, b):
    adj = np.ascontiguousarray(adj, dtype=np.float32)
    x = np.ascontiguousarray(x, dtype=np.float32)
    W = np.asarray(W, dtype=np.float32)
    b = np.asarray(b, dtype=np.float32)

    # fold the Chebyshev recursion constants into one stacked weight
    import ml_dtypes
    vs = np.concatenate([W[0] - W[2], -W[1], 2.0 * W[2]], axis=0).astype(
        ml_dtypes.bfloat16)  # [96, 32]
    bcol = b.reshape(F, 1)
    eye = np.eye(F, dtype=np.float32)
    ident2 = np.concatenate([eye, eye], axis=0).astype(ml_dtypes.bfloat16)  # [64, 32]

    if "nc" not in _cache:
        _cache["nc"] = build_nc()
    nc = _cache["nc"]

    in_maps = []
    for i in range(NCORES):
        sl = slice(i * S, (i + 1) * S)
        in_maps.append({
            "adjT": np.ascontiguousarray(adj[sl].transpose(0, 2, 1)).astype(ml_dtypes.bfloat16),
            "xT": np.ascontiguousarray(x[sl].transpose(0, 2, 1)).astype(ml_dtypes.bfloat16),
            "vs": vs,
            "bcol": bcol,
            "ident2": ident2,
        })

    trace = os.environ.get("KERNEL_TRACE") == "1"
    kw = {}
    if trace:
        _install_ntff_hook()
        import concourse.bass_utils as _bu
        _bu.upload_artifacts = lambda t: t  # no bucket in this container
        kw["tmpdir"] = os.environ.get("KERNEL_TRACE_DIR") or None
    res = run_bass_kernel_spmd(
        nc, in_maps, core_ids=list(range(NCORES)), trace=trace, **kw,
    )
    if trace and res.exec_time_ns is not None:
        print(f"HW exec time: {res.exec_time_ns} ns")

    outT = np.concatenate([res.results[i]["out"] for i in range(NCORES)], axis=0)
    return np.ascontiguousarray(outT.transpose(0, 2, 1))

